# revision 13
# baseline (speedup 1.0000x reference)
"""GemLite int4 GEMV (W4A16, group_size == in_features) on 8 Trainium2 cores.

out[b, n] = sum_k (x[b,k] * scales[k]) * (W[n,k] - zeros),  W 4-bit packed.

Strategy (column-parallel over out_features, 3584 rows per core):
- The packed int32 weights are viewed as int16 and transpose-DMA'd (xbar)
  into SBUF tiles [m16=128 partitions, n free].
- DVE dual-op tensor_scalar extracts nibble pairs in packed form:
      L16 = p & 0x0f0f ;  H16 = (p >> 4) & 0x0f0f
  Each result byte holds a weight value w in 0..15. Interpreted as
  fp8e4m3, bit patterns 0..15 are exactly w/512 (subnormals + first
  binade are linear), so the extraction output bytes ARE valid fp8
  operands; no per-element convert pass is needed.
- TensorE matmuls: stationary = permuted x*(scales*512) in fp16 [128, 4],
  moving = the fp8 byte planes [128, 512] (stride-2 free APs), accumulated
  in fp32 PSUM over 64 k-tiles.  psum = sum_k xs*w  exactly (fp22 products
  are exact: 11-bit x 4-bit significands).
- Zero-point: out = psum - zeros * sum_k xs, via one small fp32 matmul
  against a ones-vector plus a per-partition tensor_scalar subtract.

The k-order within each contraction tile is a fixed permutation of the
packed layout; x and scales are pre-permuted on the host (pure relayout)
so the matmul contracts matching elements.

Hardware/toolchain constraint honored throughout: a 64-byte TPB
instruction has exactly ONE semaphore-wait slot, so the program is
structured so Tile never needs to attach more than one wait per
instruction (single-use DMA lanes, shepherd ops to pre-observe PE ticks,
and a patched TileContext tail drain that splits its per-processor waits
across single-wait NOPs).
"""

import numpy as np

OUT_F = 28672
IN_F = 8192
B = 4
NCORES = 8
NLOC = OUT_F // NCORES          # 3584 output features per core
M16 = IN_F // 8 * 2             # 2048 int16 columns of packed weights
MTS = 16                        # m16-tiles of 128 partitions
NCHUNK = 4                      # transpose-DMA chunks
CH = MTS // NCHUNK              # m16-tiles per chunk
NT = MTS * 4                    # 64 stationary x tiles (k-tiles of 128)
NBANK = 7                       # psum chunks of 512 along n
NB = 512

_STATE = {}


def _kmap():
    """KMAP[t, p] = k index contracted by stationary tile t at partition p.

    t = mt*4 + j with j: 0=(H,bb0) 1=(H,bb1) 2=(L,bb0) 3=(L,bb1).
    Derived from the GemLite pack layout: k = (8g + i)*32 + col with
    m = g*32+col the int32 column, i the nibble index (shift 28-4i), and
    the int16/byte decomposition m16 = 2m+h, i = (6 or 7) - 4h - 2bb.
    """
    mt = np.arange(MTS)[:, None, None]          # [16, 1, 1]
    j = np.arange(4)[None, :, None]             # [1, 4, 1]
    p = np.arange(128)[None, None, :]           # [1, 1, 128]
    q, h = p >> 1, p & 1
    m = 64 * mt + q
    g, col = m // 32, m % 32
    base = np.where(j < 2, 6, 7)
    i = base - 4 * h - 2 * (j & 1)
    k = (8 * g + i) * 32 + col                  # [16, 4, 128]
    return k.reshape(NT, 128)


def _build_program():
    import concourse.bass as bass
    import concourse.tile as tile
    from concourse import mybir
    from concourse.vector_clock import ScopedClock, VectorClock

    def _split_drain_and_barrier(self, tick_clock, wait_clock):
        # Stock version puts one wait per outstanding processor on a single
        # Drain; the TPB instruction encoding has one wait slot. Split into
        # single-wait NOPs on SP.
        gc = tick_clock.global_clock
        n = len(gc)
        sp = self.nc.engines[mybir.EngineType.SP]
        for pidx in range(n):
            t = gc[pidx]
            if t > 0:
                vec = [0] * n
                vec[pidx] = t
                nop = sp.nop(nofuse=True, hint=f"tail_wait_p{pidx}")
                wait_clock.add_sem_waits(
                    nop.ins, ScopedClock({None: VectorClock(vec)})
                )
        self.nc.sync.drain()
        self.nc.all_engine_barrier()
        assert self.sems is not None
        popped = self.nc._tile_sem_poison_stack.pop()
        assert popped is self._sem_poison
        self.nc.clear_and_free_semaphores(list(self.sems.allocated().values()))
        self.nc.all_engine_barrier()

    tile.TileContext._drain_and_barrier = _split_drain_and_barrier

    dt = mybir.dt
    nc = bass.Bass()
    wq = nc.dram_tensor("wq", [NLOC, M16], dt.int16, kind="ExternalInput")
    xp = nc.dram_tensor("xp", [NT, 128, B], dt.float16, kind="ExternalInput")
    scp = nc.dram_tensor("scp", [NT, 128], dt.float16, kind="ExternalInput")
    zrep = nc.dram_tensor("zrep", [128, 1], dt.float16, kind="ExternalInput")
    out = nc.dram_tensor("out", [B, NLOC], dt.float16, kind="ExternalOutput")

    osb_cm = nc.sbuf_tensor("osb", [B, NLOC], dt.float16)
    osb = osb_cm.__enter__().ap()

    with tile.TileContext(nc) as tc:
        with (
            tc.tile_pool(name="xsb", bufs=1) as xsb,
            tc.tile_pool(name="wsb", bufs=2) as wsb,
            tc.tile_pool(name="ps", bufs=1, space=bass.MemorySpace.PSUM) as ps,
        ):
            # ---- x-side prep (all tiny) ----
            xp_sb = xsb.tile([128, NT, B], dt.float16, tag="xp")
            nc.sync.dma_start(out=xp_sb[:], in_=xp[:].rearrange("t p b -> p t b"))
            scp_sb = xsb.tile([128, NT], dt.float16, tag="scp")
            nc.sync.dma_start(out=scp_sb[:], in_=scp[:].rearrange("t p -> p t"))
            z_sb = xsb.tile([128, 1], dt.float16, tag="z")
            nc.sync.dma_start(out=z_sb[:], in_=zrep[:])

            # DVE x-prep chain. Ordering is FORCED (sync=False edges) so
            # each op needs at most one cross-proc wait: every DMA lane is
            # observed by its own dedicated DVE op, later ops wait nothing.
            from concourse.tile import add_dep_helper

            def chain(prev, cur):
                if prev is not None:
                    add_dep_helper(cur.ins, prev.ins, sync=False,
                                   reason="forced DVE order")
                return cur

            scratch = xsb.tile([1, 1], dt.float32, tag="scr")
            c0 = nc.vector.tensor_copy(scratch[:], xp_sb[0:1, 0:1, 0:1])
            zf32 = xsb.tile([128, 1], dt.float32, tag="zf32")
            c1 = chain(c0, nc.vector.tensor_copy(zf32[:], z_sb[:]))
            scp2 = xsb.tile([128, NT], dt.float16, tag="scp2")
            c2 = chain(c1, nc.vector.tensor_scalar_mul(scp2[:], scp_sb[:], 512.0))
            xsT = xsb.tile([128, NT, B], dt.float16, tag="xsT")
            prev = c2
            for b in range(B):
                prev = chain(prev, nc.vector.tensor_mul(
                    xsT[:, :, b], xp_sb[:, :, b], scp2[:]))

            # S[p, b] = sum_t xsT ; fp32, reduced along the innermost view dim
            s_sb = xsb.tile([128, B], dt.float16, tag="S")
            with nc.allow_low_precision(
                    "S rounding contributes well under 1 output ulp"):
                prev = chain(prev, nc.vector.tensor_reduce(
                    s_sb[:], xsT[:].rearrange("p t b -> p b t"),
                    axis=mybir.AxisListType.X, op=mybir.AluOpType.add,
                ))
            ones_sb = xsb.tile([128, 1], dt.float16, tag="ones")
            chain(prev, nc.vector.memset(ones_sb[:], 1.0))

            psc = ps.tile([B, 1], dt.float32, tag="psc")
            # float32r = single-pass fp22-truncated fp32 matmul (the full
            # fp32 path is a known HW-hang hazard and 4x slower)
            nc.tensor.matmul(psc[:], s_sb[:], ones_sb[:])

            psm = [ps.tile([B, NB], dt.float32, tag=f"pm{i}", name=f"pm{i}") for i in range(NBANK)]

            # ---- main pipeline over weight chunks ----
            # pt tiles are single-use (no DMA WAR wait; the transpose only
            # carries the xbar-FIFO wait).  H is extracted in place over pt.
            # L tiles are double-buffered; a shepherd op ordered before the
            # L extraction absorbs the PE tick of the reused slot so the
            # extraction itself keeps a single (DMA lane) wait.
            l_t = [None] * NCHUNK
            for ci in range(NCHUNK):
                pt = wsb.tile([128, CH, NLOC], dt.int16, tag=f"pt{ci}",
                              name=f"pt{ci}", bufs=1)
                nc.sync.dma_start_transpose(
                    out=pt[:], in_=wq[:, ci * CH * 128:(ci + 1) * CH * 128]
                )
                prev = None
                if ci >= 2:
                    # WRITE the corner element of the retiring L tile: the
                    # write-after-read dep makes this DVE op absorb the PE
                    # last-reader tick; the slot wait of the reusing extract
                    # then collapses to this op's (same-engine) tick
                    prev = nc.vector.memset(
                        l_t[ci - 2][0:1, CH - 1:CH, NLOC - 1:NLOC], 0)
                # pre-observe the transpose-DMA lane on DVE so the extract's
                # input-readiness wait is elided (slot wait stays its only one)
                touch = nc.vector.tensor_copy(scratch[:], pt[0:1, 0:1, 0:1])
                if prev is not None:
                    add_dep_helper(touch.ins, prev.ins, sync=False,
                                   reason="order shepherd before touch")
                lt = wsb.tile([128, CH, NLOC], dt.int16, tag="lt", name="lt")
                lx = nc.vector.tensor_scalar(
                    lt[:], pt[:], 0x0F0F, None,
                    op0=mybir.AluOpType.bitwise_and,
                )
                add_dep_helper(lx.ins, touch.ins, sync=False,
                               reason="order touch before L-extract")
                nc.vector.tensor_scalar(
                    pt[:], pt[:], 4, 0x0F0F,
                    op0=mybir.AluOpType.logical_shift_right,
                    op1=mybir.AluOpType.bitwise_and,
                )
                l_t[ci] = lt

                h8 = pt[:].bitcast(dt.float8e4).rearrange(
                    "p c (n two) -> p c two n", two=2)
                l8 = lt[:].bitcast(dt.float8e4).rearrange(
                    "p c (n two) -> p c two n", two=2)
                for jj in range(CH):
                    mt = ci * CH + jj
                    for j in range(4):
                        src = h8 if j < 2 else l8
                        bb = j & 1
                        t = mt * 4 + j
                        first = mt == 0 and j == 0
                        last = mt == MTS - 1 and j == 3
                        for nb in range(NBANK):
                            nc.tensor.matmul(
                                psm[nb][:],
                                xsT[:, t, :],
                                src[:, jj, bb, nb * NB:(nb + 1) * NB],
                                start=first,
                                stop=last,
                            )

            # ---- epilogue: corr = zeros/512 * psc ; out = psm - corr ----
            corr = xsb.tile([B, 1], dt.float32, tag="corr")
            nc.vector.tensor_scalar(
                corr[:], psc[:], zf32[0:B, 0:1], 1.0 / 512.0,
                op0=mybir.AluOpType.mult, op1=mybir.AluOpType.mult,
            )
            for nb in range(NBANK):
                nc.vector.tensor_scalar(
                    osb[:, nb * NB:(nb + 1) * NB], psm[nb][:], corr[:],
                    None, op0=mybir.AluOpType.subtract,
                )

    with nc.semaphore("osem") as osem:
        nc.sync.dma_start(out=out[:], in_=osb).then_inc(osem, 16)
        nc.sync.wait_ge(osem, 16)

    nc.finalize()

    # static guard: the TPB encoding has one wait slot per instruction
    bad = [
        (name, len(i.sync_info.on_wait))
        for name, i in nc.inst_map.items()
        if i.sync_info and i.sync_info.on_wait and len(i.sync_info.on_wait) > 1
    ]
    assert not bad, f"multi-wait instructions would fail codegen: {bad}"
    return nc


def kernel(x, W_q_packed, scales, zeros):
    if "nc" not in _STATE:
        _STATE["nc"] = _build_program()
        _STATE["kmap"] = _kmap()
    nc = _STATE["nc"]
    kmap = _STATE["kmap"]

    from concourse.bass_utils import run_bass_kernel_spmd

    x = np.asarray(x, dtype=np.float16)
    scales = np.asarray(scales, dtype=np.float16)
    wp = np.ascontiguousarray(np.asarray(W_q_packed, dtype=np.int32))
    z = np.float16(np.asarray(zeros))

    xp = np.ascontiguousarray(x[:, kmap].transpose(1, 2, 0))   # [64, 128, 4]
    scp = np.ascontiguousarray(scales[kmap])                   # [64, 128]
    zrep = np.full((128, 1), z, dtype=np.float16)

    in_maps = []
    for c in range(NCORES):
        in_maps.append({
            "wq": wp[c * NLOC:(c + 1) * NLOC].view(np.int16),
            "xp": xp,
            "scp": scp,
            "zrep": zrep,
        })
    res = run_bass_kernel_spmd(nc, in_maps, core_ids=list(range(NCORES)))
    return np.concatenate(
        [res.results[c]["out"] for c in range(NCORES)], axis=1
    ).astype(np.float16)


# revision 19
# speedup vs baseline: 1.1569x; 1.1569x over previous
"""GemLite int4 GEMV (W4A16, group_size == in_features) on 8 Trainium2 cores.

out[b, n] = sum_k (x[b,k] * scales[k]) * (W[n,k] - zeros),  W 4-bit packed.

Strategy (column-parallel over out_features, 3584 rows per core):
- The packed int32 weights are viewed as int16 and transpose-DMA'd (xbar)
  into SBUF tiles [m16=128 partitions x 16 tiles, n free].  Chunking is
  along n with full-row (4KB contiguous) source slabs, which keeps the
  xbar transpose on its fast path.
- DVE dual-op tensor_scalar extracts nibble pairs in packed form:
      L16 = p & 0x0f0f ;  H16 = (p >> 4) & 0x0f0f   (H in place)
  Each result byte holds a weight value w in 0..15. Interpreted as
  fp8e4m3, bit patterns 0..15 are exactly w/512 (subnormals + the first
  binade are linear), so the extraction output bytes ARE valid fp8
  matmul operands; no per-element convert pass is needed.
- TensorE matmuls: stationary = permuted x*(scales*512) in fp16 [128, 4],
  moving = fp8 byte planes [128, 512] (stride-2 free APs), accumulated in
  fp32 PSUM over 64 k-tiles per n-chunk.  psum = sum_k xs*w exactly
  (11-bit x 4-bit significand products are exact in the fp22 datapath).
- Zero-point: out = psum - zeros * sum_k xs via one tiny matmul against a
  ones-vector and a per-partition tensor_scalar subtract.

The k-order within each contraction tile is a fixed permutation of the
packed layout; x and scales are pre-permuted on the host (pure relayout)
so the matmul contracts matching elements.

Toolchain constraint honored throughout: a 64-byte TPB instruction has
exactly ONE semaphore-wait slot, so the program is structured so Tile
never needs >1 wait per instruction (single-use DMA lanes, shepherd ops
that pre-absorb cross-engine ticks, forced same-engine ordering edges,
and a patched TileContext tail drain that splits its per-processor waits
across single-wait NOPs).
"""

import numpy as np

OUT_F = 28672
IN_F = 8192
B = 4
NCORES = 8
NLOC = OUT_F // NCORES          # 3584 output features per core
M16 = IN_F // 8 * 2             # 2048 int16 columns of packed weights
MTS = 16                        # k-tiles of 128 partitions (m16 tiles)
NT = MTS * 4                    # 64 stationary x tiles
NB = 512                        # n-chunk width == psum bank capacity
NCHUNK = NLOC // NB             # 7 n-chunks
WBUFS = 3                       # pipeline depth for weight tiles
NWARM = 24                      # PE warm-up matmuls

_STATE = {}


def _kmap():
    """KMAP[t, p] = k index contracted by stationary tile t at partition p.

    t = mt*4 + j with j: 0=(H,bb0) 1=(H,bb1) 2=(L,bb0) 3=(L,bb1).
    Derived from the GemLite pack layout: k = (8g + i)*32 + col with
    m = g*32+col the int32 column, i the nibble index (shift 28-4i), and
    the int16/byte decomposition m16 = 2m+h, i = (6 or 7) - 4h - 2bb.
    """
    mt = np.arange(MTS)[:, None, None]
    j = np.arange(4)[None, :, None]
    p = np.arange(128)[None, None, :]
    q, h = p >> 1, p & 1
    m = 64 * mt + q
    g, col = m // 32, m % 32
    base = np.where(j < 2, 6, 7)
    i = base - 4 * h - 2 * (j & 1)
    k = (8 * g + i) * 32 + col
    return k.reshape(NT, 128)


def _build_program():
    import concourse.bass as bass
    import concourse.tile as tile
    from concourse import mybir
    from concourse.tile import add_dep_helper
    from concourse.vector_clock import ScopedClock, VectorClock

    def _split_drain_and_barrier(self, tick_clock, wait_clock):
        # Stock version puts one wait per outstanding processor on a single
        # Drain; the TPB encoding has one wait slot. Split into single-wait
        # NOPs on SP.
        gc = tick_clock.global_clock
        n = len(gc)
        sp = self.nc.engines[mybir.EngineType.SP]
        for pidx in range(n):
            t = gc[pidx]
            if t > 0:
                vec = [0] * n
                vec[pidx] = t
                nop = sp.nop(nofuse=True, hint=f"tail_wait_p{pidx}")
                wait_clock.add_sem_waits(
                    nop.ins, ScopedClock({None: VectorClock(vec)})
                )
        self.nc.sync.drain()
        self.nc.all_engine_barrier()
        assert self.sems is not None
        popped = self.nc._tile_sem_poison_stack.pop()
        assert popped is self._sem_poison
        self.nc.clear_and_free_semaphores(list(self.sems.allocated().values()))
        self.nc.all_engine_barrier()

    tile.TileContext._drain_and_barrier = _split_drain_and_barrier

    dt = mybir.dt
    nc = bass.Bass()
    wq = nc.dram_tensor("wq", [NLOC, M16], dt.int16, kind="ExternalInput")
    # xmeta[p, 0:NT*B] = x[b, KMAP[t,p]] at t*B+b ; [NT*B : NT*B+NT] =
    # scales[KMAP[t,p]] ; last column = zeros scalar (replicated)
    XM = NT * B + NT + 1
    xmeta = nc.dram_tensor("xmeta", [128, XM], dt.float16, kind="ExternalInput")
    out = nc.dram_tensor("out", [B, NLOC], dt.float16, kind="ExternalOutput")

    osb_cm = nc.sbuf_tensor("osb", [B, NLOC], dt.float16)
    osb = osb_cm.__enter__().ap()

    with tile.TileContext(nc) as tc:
        with (
            tc.tile_pool(name="xsb", bufs=1) as xsb,
            tc.tile_pool(name="wsb", bufs=WBUFS) as wsb,
            tc.tile_pool(name="ps", bufs=1, space=bass.MemorySpace.PSUM) as ps,
        ):
            def chain(prev, cur):
                if prev is not None:
                    add_dep_helper(cur.ins, prev.ins, sync=False,
                                   reason="forced order")
                return cur

            # ---- PE warm-up: release the HAM clock gate while DMAs run ----
            warm = xsb.tile([128, NB], dt.float16, tag="warm")
            nc.vector.memset(warm[:], 0.0)
            psm = [ps.tile([B, NB], dt.float32, tag=f"pm{i}", name=f"pm{i}")
                   for i in range(NCHUNK)]
            for wi in range(NWARM):
                nc.tensor.matmul(psm[wi % NCHUNK][:], warm[:, 0:B], warm[:],
                                 start=True, stop=True)

            # ---- x-side prep (tiny); forced into one DVE chain ----
            xm_sb = xsb.tile([128, XM], dt.float16, tag="xm")
            nc.sync.dma_start(out=xm_sb[:], in_=xmeta[:])
            xp_v = xm_sb[:, 0:NT * B].rearrange("p (t b) -> p t b", b=B)
            scp_v = xm_sb[:, NT * B:NT * B + NT]
            z_v = xm_sb[:, XM - 1:XM]

            scp2 = xsb.tile([128, NT], dt.float16, tag="scp2")
            c = chain(None, nc.vector.tensor_scalar_mul(scp2[:], scp_v, 512.0))
            xsT = xsb.tile([128, NT, B], dt.float16, tag="xsT")
            for b in range(B):
                c = chain(c, nc.vector.tensor_mul(
                    xsT[:, :, b], xp_v[:, :, b], scp2[:]))
            s_sb = xsb.tile([128, B], dt.float16, tag="S")
            with nc.allow_low_precision(
                    "S rounding contributes well under 1 output ulp"):
                c = chain(c, nc.vector.tensor_reduce(
                    s_sb[:], xsT[:].rearrange("p t b -> p b t"),
                    axis=mybir.AxisListType.X, op=mybir.AluOpType.add,
                ))
            # rhs vector = -zeros/512 replicated: psc = -zeros * sum_k xs
            zv = xsb.tile([128, 1], dt.float16, tag="zv")
            c = chain(c, nc.vector.tensor_scalar_mul(zv[:], z_v, -1.0 / 512.0))

            psc = ps.tile([B, 1], dt.float32, tag="psc")
            nc.tensor.matmul(psc[:], s_sb[:], zv[:])
            # stage the correction into SBUF for the ACT epilogues' bias AP
            cors = xsb.tile([B, 1], dt.float32, tag="cors")
            nc.vector.tensor_copy(cors[:], psc[:])
            # ACT pre-observes the DVE tick so each epilogue carries only
            # its PE wait
            scr2 = xsb.tile([1, 1], dt.float32, tag="scr2")
            nc.scalar.activation(scr2[:], cors[0:1, 0:1],
                                 mybir.ActivationFunctionType.Copy)

            # ---- main pipeline over n-chunks ----
            scratch = xsb.tile([1, 1], dt.float32, tag="scr")
            l_t = [None] * NCHUNK
            for ci in range(NCHUNK):
                pt = wsb.tile([128, MTS, NB], dt.int16, tag=f"pt{ci}", name=f"pt{ci}", bufs=1)
                # full-row source slab: 4KB-contiguous reads -> fast xbar path
                nc.sync.dma_start_transpose(
                    out=pt[:], in_=wq[ci * NB:(ci + 1) * NB, :]
                )
                prev = None
                if ci >= WBUFS:
                    # write the corner element of the retiring L tile: this
                    # DVE op absorbs the PE last-reader tick so the
                    # slot-reusing extract below keeps a single wait
                    prev = nc.vector.memset(
                        l_t[ci - WBUFS][0:1, MTS - 1:MTS, NB - 1:NB], 0)
                # pre-observe the transpose lane on DVE (elides the
                # extract's input wait; its slot wait stays the only one)
                touch = chain(prev, nc.vector.tensor_copy(
                    scratch[:], pt[0:1, 0:1, 0:1]))
                lt = wsb.tile([128, MTS, NB], dt.int16, tag="lt", name="lt")
                lx = chain(touch, nc.vector.tensor_scalar(
                    lt[:], pt[:], 0x0F0F, None,
                    op0=mybir.AluOpType.bitwise_and,
                ))
                hx = chain(lx, nc.vector.tensor_scalar(
                    pt[:], pt[:], 4, 0x0F0F,
                    op0=mybir.AluOpType.logical_shift_right,
                    op1=mybir.AluOpType.bitwise_and,
                ))
                l_t[ci] = lt

                h8 = pt[:].bitcast(dt.float8e4).rearrange(
                    "p m (n two) -> p m two n", two=2)
                l8 = lt[:].bitcast(dt.float8e4).rearrange(
                    "p m (n two) -> p m two n", two=2)
                for mt in range(MTS):
                    for j in range(4):
                        src = h8 if j < 2 else l8
                        nc.tensor.matmul(
                            psm[ci][:],
                            xsT[:, mt * 4 + j, :],
                            src[:, mt, j & 1, :],
                            start=(mt == 0 and j == 0),
                            stop=(mt == MTS - 1 and j == 3),
                        )
                # epilogue on ScalarE (own queue -> never stalls DVE
                # extraction): out = psum + (-zeros*sum xs) per partition
                nc.scalar.activation(
                    osb[:, ci * NB:(ci + 1) * NB], psm[ci][:],
                    mybir.ActivationFunctionType.Identity,
                    bias=cors[0:B, 0:1], scale=1.0,
                )

    with nc.semaphore("osem") as osem:
        nc.sync.dma_start(out=out[:], in_=osb).then_inc(osem, 16)
        nc.sync.wait_ge(osem, 16)

    nc.finalize()

    bad = [
        (name, len(i.sync_info.on_wait))
        for name, i in nc.inst_map.items()
        if i.sync_info and i.sync_info.on_wait and len(i.sync_info.on_wait) > 1
    ]
    assert not bad, f"multi-wait instructions would fail codegen: {bad}"
    return nc


def _prep_inputs(x, W_q_packed, scales, zeros, kmap):
    x = np.asarray(x, dtype=np.float16)
    scales = np.asarray(scales, dtype=np.float16)
    wp = np.ascontiguousarray(np.asarray(W_q_packed, dtype=np.int32))
    z = np.float16(np.asarray(zeros))

    XM = NT * B + NT + 1
    xmeta = np.empty((128, XM), dtype=np.float16)
    # xp: x[:, kmap] is [4, 64, 128]; want [p, t*B + b]
    xmeta[:, 0:NT * B] = x[:, kmap].transpose(2, 1, 0).reshape(128, NT * B)
    xmeta[:, NT * B:NT * B + NT] = scales[kmap].T
    xmeta[:, XM - 1] = z

    in_maps = []
    for c in range(NCORES):
        in_maps.append({
            "wq": wp[c * NLOC:(c + 1) * NLOC].view(np.int16),
            "xmeta": xmeta,
        })
    return in_maps


def kernel(x, W_q_packed, scales, zeros):
    if "nc" not in _STATE:
        _STATE["nc"] = _build_program()
        _STATE["kmap"] = _kmap()
    nc = _STATE["nc"]

    from concourse.bass_utils import run_bass_kernel_spmd

    in_maps = _prep_inputs(x, W_q_packed, scales, zeros, _STATE["kmap"])
    res = run_bass_kernel_spmd(nc, in_maps, core_ids=list(range(NCORES)))
    return np.concatenate(
        [res.results[c]["out"] for c in range(NCORES)], axis=1
    ).astype(np.float16)


# revision 22
# speedup vs baseline: 1.5651x; 1.3528x over previous
"""GemLite int4 GEMV (W4A16, group_size == in_features) on 8 Trainium2 cores.

out[b, n] = sum_k (x[b,k] * scales[k]) * (W[n,k] - zeros),  W 4-bit packed.

Strategy (column-parallel over out_features, 3584 rows per core):
- The packed int32 weights are viewed as int16 and transpose-DMA'd (xbar)
  into SBUF tiles [m16=128 partitions x 16 tiles, n free].  Chunking is
  along n with full-row (4KB contiguous) source slabs, which keeps the
  xbar transpose on its fast path.
- DVE dual-op tensor_scalar extracts nibble pairs in packed form:
      L16 = p & 0x0f0f ;  H16 = (p >> 4) & 0x0f0f   (H in place)
  Each result byte holds a weight value w in 0..15. Interpreted as
  fp8e4m3, bit patterns 0..15 are exactly w/512 (subnormals + the first
  binade are linear), so the extraction output bytes ARE valid fp8
  matmul operands; no per-element convert pass is needed.
- TensorE matmuls: stationary = permuted x*(scales*512) in fp16 [128, 4],
  moving = fp8 byte planes [128, 512] (stride-2 free APs), accumulated in
  fp32 PSUM over 64 k-tiles per n-chunk.  psum = sum_k xs*w exactly
  (11-bit x 4-bit significand products are exact in the fp22 datapath).
- Zero-point: out = psum - zeros * sum_k xs via one tiny matmul against a
  ones-vector and a per-partition tensor_scalar subtract.

The k-order within each contraction tile is a fixed permutation of the
packed layout; x and scales are pre-permuted on the host (pure relayout)
so the matmul contracts matching elements.

Toolchain constraint honored throughout: a 64-byte TPB instruction has
exactly ONE semaphore-wait slot, so the program is structured so Tile
never needs >1 wait per instruction (single-use DMA lanes, shepherd ops
that pre-absorb cross-engine ticks, forced same-engine ordering edges,
and a patched TileContext tail drain that splits its per-processor waits
across single-wait NOPs).
"""

import numpy as np

OUT_F = 28672
IN_F = 8192
B = 4
NCORES = 8
NLOC = OUT_F // NCORES          # 3584 output features per core
M16 = IN_F // 8 * 2             # 2048 int16 columns of packed weights
MTS = 16                        # k-tiles of 128 partitions (m16 tiles)
NT = MTS * 4                    # 64 stationary x tiles
NB = 512                        # n-chunk width == psum bank capacity
NCHUNK = NLOC // NB             # 7 n-chunks
WBUFS = 3                       # pipeline depth for weight tiles
NWARM = 24                      # PE warm-up matmuls
COLT = 2                        # concurrent PE column-group streams
NW = NB // COLT                 # n-width per stream matmul

_STATE = {}


def _kmap():
    """KMAP[t, p] = k index contracted by stationary tile t at partition p.

    t = mt*4 + j with j: 0=(H,bb0) 1=(H,bb1) 2=(L,bb0) 3=(L,bb1).
    Derived from the GemLite pack layout: k = (8g + i)*32 + col with
    m = g*32+col the int32 column, i the nibble index (shift 28-4i), and
    the int16/byte decomposition m16 = 2m+h, i = (6 or 7) - 4h - 2bb.
    """
    mt = np.arange(MTS)[:, None, None]
    j = np.arange(4)[None, :, None]
    p = np.arange(128)[None, None, :]
    q, h = p >> 1, p & 1
    m = 64 * mt + q
    g, col = m // 32, m % 32
    base = np.where(j < 2, 6, 7)
    i = base - 4 * h - 2 * (j & 1)
    k = (8 * g + i) * 32 + col
    return k.reshape(NT, 128)


def _build_program():
    import concourse.bass as bass
    import concourse.tile as tile
    from concourse import mybir
    from concourse.tile import add_dep_helper
    from concourse.vector_clock import ScopedClock, VectorClock

    def _split_drain_and_barrier(self, tick_clock, wait_clock):
        # Stock version puts one wait per outstanding processor on a single
        # Drain; the TPB encoding has one wait slot. Split into single-wait
        # NOPs on SP.
        gc = tick_clock.global_clock
        n = len(gc)
        sp = self.nc.engines[mybir.EngineType.SP]
        for pidx in range(n):
            t = gc[pidx]
            if t > 0:
                vec = [0] * n
                vec[pidx] = t
                nop = sp.nop(nofuse=True, hint=f"tail_wait_p{pidx}")
                wait_clock.add_sem_waits(
                    nop.ins, ScopedClock({None: VectorClock(vec)})
                )
        self.nc.sync.drain()
        self.nc.all_engine_barrier()
        assert self.sems is not None
        popped = self.nc._tile_sem_poison_stack.pop()
        assert popped is self._sem_poison
        self.nc.clear_and_free_semaphores(list(self.sems.allocated().values()))
        self.nc.all_engine_barrier()

    tile.TileContext._drain_and_barrier = _split_drain_and_barrier

    dt = mybir.dt
    nc = bass.Bass()
    wq = nc.dram_tensor("wq", [NLOC, M16], dt.int16, kind="ExternalInput")
    # xmeta[p, 0:NT*B] = x[b, KMAP[t,p]] at t*B+b ; [NT*B : NT*B+NT] =
    # scales[KMAP[t,p]] ; last column = zeros scalar (replicated)
    XM = NT * B + NT + 1
    xmeta = nc.dram_tensor("xmeta", [128, XM], dt.float16, kind="ExternalInput")
    out = nc.dram_tensor("out", [B, NLOC], dt.float16, kind="ExternalOutput")

    osb_cm = nc.sbuf_tensor("osb", [32 * (COLT - 1) + B, NCHUNK * NW],
                            dt.float16)
    osb = osb_cm.__enter__().ap()

    with tile.TileContext(nc) as tc:
        with (
            tc.tile_pool(name="xsb", bufs=1) as xsb,
            tc.tile_pool(name="wsb", bufs=WBUFS) as wsb,
            tc.tile_pool(name="ps", bufs=1, space=bass.MemorySpace.PSUM) as ps,
        ):
            def chain(prev, cur):
                if prev is not None:
                    add_dep_helper(cur.ins, prev.ins, sync=False,
                                   reason="forced order")
                return cur

            # ---- PE warm-up: release the HAM clock gate while DMAs run ----
            warm = xsb.tile([128, NB], dt.float16, tag="warm")
            nc.vector.memset(warm[:], 0.0)
            psm = [ps.tile([128, NB], dt.float32, tag=f"pm{i}", name=f"pm{i}")
                   for i in range(NCHUNK)]
            for wi in range(NWARM):
                for js in range(COLT):
                    nc.tensor.matmul(
                        psm[wi % NCHUNK][32 * js:32 * js + B, 0:NW],
                        warm[:, 0:B], warm[:, 0:NW],
                        start=True, stop=True,
                        tile_position=(0, 32 * js),
                    )

            # ---- x-side prep (tiny); forced into one DVE chain ----
            xm_sb = xsb.tile([128, XM], dt.float16, tag="xm")
            nc.sync.dma_start(out=xm_sb[:], in_=xmeta[:])
            xp_v = xm_sb[:, 0:NT * B].rearrange("p (t b) -> p t b", b=B)
            scp_v = xm_sb[:, NT * B:NT * B + NT]
            z_v = xm_sb[:, XM - 1:XM]

            scp2 = xsb.tile([128, NT], dt.float16, tag="scp2")
            c = chain(None, nc.vector.tensor_scalar_mul(scp2[:], scp_v, 512.0))
            xsT = xsb.tile([128, NT, B], dt.float16, tag="xsT")
            for b in range(B):
                c = chain(c, nc.vector.tensor_mul(
                    xsT[:, :, b], xp_v[:, :, b], scp2[:]))
            s_sb = xsb.tile([128, B], dt.float16, tag="S")
            with nc.allow_low_precision(
                    "S rounding contributes well under 1 output ulp"):
                c = chain(c, nc.vector.tensor_reduce(
                    s_sb[:], xsT[:].rearrange("p t b -> p b t"),
                    axis=mybir.AxisListType.X, op=mybir.AluOpType.add,
                ))
            # rhs vector = -zeros/512 replicated: psc = -zeros * sum_k xs
            zv = xsb.tile([128, 1], dt.float16, tag="zv")
            c = chain(c, nc.vector.tensor_scalar_mul(zv[:], z_v, -1.0 / 512.0))

            psc = ps.tile([128, 1], dt.float32, tag="psc")
            for js in range(COLT):
                nc.tensor.matmul(psc[32 * js:32 * js + B, :], s_sb[:], zv[:],
                                 tile_position=(0, 32 * js))
            # stage the correction into SBUF for the ACT epilogues' bias AP
            cors = xsb.tile([128, 1], dt.float32, tag="cors")
            for js in range(COLT):
                nc.vector.tensor_copy(cors[32 * js:32 * js + B, :],
                                      psc[32 * js:32 * js + B, :])
            # ACT pre-observes the DVE tick so each epilogue carries only
            # its PE wait
            scr2 = xsb.tile([1, 1], dt.float32, tag="scr2")
            nc.scalar.activation(
                scr2[:], cors[32 * (COLT - 1):32 * (COLT - 1) + 1, 0:1],
                mybir.ActivationFunctionType.Copy)

            # ---- main pipeline over n-chunks ----
            scratch = xsb.tile([1, 1], dt.float32, tag="scr")
            l_t = [None] * NCHUNK
            for ci in range(NCHUNK):
                pt = wsb.tile([128, MTS, NB], dt.int16, tag=f"pt{ci}", name=f"pt{ci}", bufs=1)
                # full-row source slab: 4KB-contiguous reads -> fast xbar path
                nc.sync.dma_start_transpose(
                    out=pt[:], in_=wq[ci * NB:(ci + 1) * NB, :]
                )
                prev = None
                if ci >= WBUFS:
                    # write the corner element of the retiring L tile: this
                    # DVE op absorbs the PE last-reader tick so the
                    # slot-reusing extract below keeps a single wait
                    prev = nc.vector.memset(
                        l_t[ci - WBUFS][0:1, MTS - 1:MTS, NB - 1:NB], 0)
                # pre-observe the transpose lane on DVE (elides the
                # extract's input wait; its slot wait stays the only one)
                touch = chain(prev, nc.vector.tensor_copy(
                    scratch[:], pt[0:1, 0:1, 0:1]))
                lt = wsb.tile([128, MTS, NB], dt.int16, tag="lt", name="lt")
                lx = chain(touch, nc.vector.tensor_scalar(
                    lt[:], pt[:], 0x0F0F, None,
                    op0=mybir.AluOpType.bitwise_and,
                ))
                hx = chain(lx, nc.vector.tensor_scalar(
                    pt[:], pt[:], 4, 0x0F0F,
                    op0=mybir.AluOpType.logical_shift_right,
                    op1=mybir.AluOpType.bitwise_and,
                ))
                l_t[ci] = lt

                h8 = pt[:].bitcast(dt.float8e4).rearrange(
                    "p m (n two) -> p m two n", two=2)
                l8 = lt[:].bitcast(dt.float8e4).rearrange(
                    "p m (n two) -> p m two n", two=2)
                for mt in range(MTS):
                    for j in range(4):
                        src = h8 if j < 2 else l8
                        for js in range(COLT):
                            nc.tensor.matmul(
                                psm[ci][32 * js:32 * js + B,
                                        js * NW:(js + 1) * NW],
                                xsT[:, mt * 4 + j, :],
                                src[:, mt, j & 1, js * NW:(js + 1) * NW],
                                start=(mt == 0 and j == 0),
                                stop=(mt == MTS - 1 and j == 3),
                                tile_position=(0, 32 * js),
                            )
                # epilogue on ScalarE (own queue -> never stalls DVE
                # extraction): out = psum + (-zeros*sum xs) per partition;
                # stream js lands in osb partitions 32*js+b
                for js in range(COLT):
                    nc.scalar.activation(
                        osb[32 * js:32 * js + B,
                            ci * NW:(ci + 1) * NW],
                        psm[ci][32 * js:32 * js + B, js * NW:(js + 1) * NW],
                        mybir.ActivationFunctionType.Identity,
                        bias=cors[32 * js:32 * js + B, 0:1], scale=1.0,
                    )

    outv = out[:].rearrange("b (ci js q) -> b ci js q", js=COLT, q=NW)
    with nc.semaphore("osem") as osem:
        for js in range(COLT):
            nc.sync.dma_start(
                out=outv[:, :, js, :],
                in_=osb[32 * js:32 * js + B, :].rearrange(
                    "b (ci q) -> b ci q", q=NW),
            ).then_inc(osem, 16)
        nc.sync.wait_ge(osem, 16 * COLT)

    nc.finalize()

    bad = [
        (name, len(i.sync_info.on_wait))
        for name, i in nc.inst_map.items()
        if i.sync_info and i.sync_info.on_wait and len(i.sync_info.on_wait) > 1
    ]
    assert not bad, f"multi-wait instructions would fail codegen: {bad}"
    return nc


def _prep_inputs(x, W_q_packed, scales, zeros, kmap):
    x = np.asarray(x, dtype=np.float16)
    scales = np.asarray(scales, dtype=np.float16)
    wp = np.ascontiguousarray(np.asarray(W_q_packed, dtype=np.int32))
    z = np.float16(np.asarray(zeros))

    XM = NT * B + NT + 1
    xmeta = np.empty((128, XM), dtype=np.float16)
    # xp: x[:, kmap] is [4, 64, 128]; want [p, t*B + b]
    xmeta[:, 0:NT * B] = x[:, kmap].transpose(2, 1, 0).reshape(128, NT * B)
    xmeta[:, NT * B:NT * B + NT] = scales[kmap].T
    xmeta[:, XM - 1] = z

    in_maps = []
    for c in range(NCORES):
        in_maps.append({
            "wq": wp[c * NLOC:(c + 1) * NLOC].view(np.int16),
            "xmeta": xmeta,
        })
    return in_maps


def kernel(x, W_q_packed, scales, zeros):
    if "nc" not in _STATE:
        _STATE["nc"] = _build_program()
        _STATE["kmap"] = _kmap()
    nc = _STATE["nc"]

    from concourse.bass_utils import run_bass_kernel_spmd

    in_maps = _prep_inputs(x, W_q_packed, scales, zeros, _STATE["kmap"])
    res = run_bass_kernel_spmd(nc, in_maps, core_ids=list(range(NCORES)))
    return np.concatenate(
        [res.results[c]["out"] for c in range(NCORES)], axis=1
    ).astype(np.float16)
